# revision 18
# baseline (speedup 1.0000x reference)
"""Meet-in-the-middle DP, fp16 two-chain interleaved variant.

Per core: 256 samples as 128 partitions x 2 sample-pairs. Free-dim layout
per DP row (host-packed, fp16, guard columns = 30000 instead of a +512
bias so fp16 keeps full precision on the small DP values):

  [ FS0(64) G FS1(64) G | BS1(64) G BS0(64) G ]  = 260 cols
    chain A (130)          chain B (130)

F chains walk rows 0..31; B chains walk rows 63..32 on a both-axes-flipped
view (pre-flipped on the host, so every op is a plain forward scan). Per
row, chain A and chain B each run one shifted-min TT (fp16, 2x DVE mode)
and one min-add scan, interleaved TT_A,TT_B,scan_A,scan_B so each
instruction's semaphore wait (95ns update latency) is hidden under the
other chain's execution — the DVE engine runs gap-free. The scan carry is
fp32 in hardware regardless of operand dtype, so fp16 storage only costs
one half-ulp requantization per row (validated ~2e-3 max rel err).
Input DMA is host-packed so every chunk is one contiguous >=520B run per
partition (no sub-512B descriptor penalty).
"""

import sys

import numpy as np

sys.path.insert(0, "/opt/trn_rl_repo")

import concourse.bacc as bacc
import concourse.mybir as mybir
import concourse.tile as tile
from concourse.bass_utils import run_bass_kernel_spmd

P = 128
Q = 2
H = 64
W = 64
HH = H // 2
SLOT = W + 1          # 64 data cols + guard
NSLOT = 4
L = NSLOT * SLOT      # 260
CW = 2 * SLOT         # 130: per-chain layout width
AW = CW - 1           # 129: active op width (trailing guard col never read)
NB_CORE = P * Q
N_CORES = 8
GUARD = 30000.0
BIG = 1.0e9
F16 = mybir.dt.float16
F32 = mybir.dt.float32
MIN = mybir.AluOpType.min
ADD = mybir.AluOpType.add

# input DMA chunk boundaries (rows): small first chunks hide startup latency
CHUNKS = (0, 1, 2, 4, 8, 16, 32)

_CACHE = {}


def _build():
    nc = bacc.Bacc("TRN2", debug=False, target_bir_lowering=False,
                   num_devices=N_CORES)
    img_d = nc.dram_tensor("images", [P, HH, L], F16,
                           kind="ExternalInput").ap()
    out_d = nc.dram_tensor("out", [P, Q], F32, kind="ExternalOutput").ap()

    with tile.TileContext(nc) as tc:
        with tc.tile_pool(name="img", bufs=1) as imgp, \
             tc.tile_pool(name="state", bufs=1) as statep, \
             tc.tile_pool(name="work", bufs=2) as workp:
            imgT = imgp.tile([P, HH, L], F16)
            za = statep.tile([P, CW + 1], F16)    # [guard | cols 0..129]
            zb = statep.tile([P, CW + 1], F16)    # [guard | cols 130..259]
            c0a = statep.tile([P, AW], F16)
            c0b = statep.tile([P, AW], F16)
            cc = statep.tile([P, Q, W], F16)      # seam scratch
            t2 = statep.tile([P, Q, W], F16)
            red = statep.tile([P, Q], F32)

            # no-dependency prep first: runs during the DMA fill latency
            nc.vector.memset(za[:, 0:1], GUARD)
            nc.vector.memset(zb[:, 0:1], GUARD)
            nc.vector.memset(c0a[:], GUARD)
            nc.vector.memset(c0b[:], GUARD)

            for i, (a, b) in enumerate(zip(CHUNKS[:-1], CHUNKS[1:])):
                eng = nc.sync if i % 2 == 0 else nc.scalar
                eng.dma_start(out=imgT[:, a:b, :], in_=img_d[:, a:b, :])

            # row 0 seed: c0 = GUARD except -img_start/2 at slot starts
            nc.vector.tensor_scalar_mul(c0a[:, 0:AW:SLOT],
                                        imgT[:, 0, 0:AW:SLOT], -0.5)
            nc.vector.tensor_scalar_mul(c0b[:, 0:AW:SLOT],
                                        imgT[:, 0, CW:CW + AW:SLOT], -0.5)
            nc.vector.tensor_tensor_scan(
                out=za[:, 1:1 + AW], data0=c0a[:], data1=imgT[:, 0, 0:AW],
                initial=BIG, op0=MIN, op1=ADD)
            nc.vector.tensor_tensor_scan(
                out=zb[:, 1:1 + AW], data0=c0b[:],
                data1=imgT[:, 0, CW:CW + AW],
                initial=BIG, op0=MIN, op1=ADD)

            for r in range(1, HH):
                ma = workp.tile([P, AW], F16, tag="ma", name=f"ma_{r}")
                mb = workp.tile([P, AW], F16, tag="mb", name=f"mb_{r}")
                nc.vector.tensor_tensor(out=ma[:], in0=za[:, 1:1 + AW],
                                        in1=za[:, 0:AW], op=MIN)
                nc.vector.tensor_tensor(out=mb[:], in0=zb[:, 1:1 + AW],
                                        in1=zb[:, 0:AW], op=MIN)
                nc.vector.tensor_tensor_scan(
                    out=za[:, 1:1 + AW], data0=ma[:], data1=imgT[:, r, 0:AW],
                    initial=BIG, op0=MIN, op1=ADD)
                nc.vector.tensor_tensor_scan(
                    out=zb[:, 1:1 + AW], data0=mb[:],
                    data1=imgT[:, r, CW:CW + AW],
                    initial=BIG, op0=MIN, op1=ADD)

            # seam at rows 31/32, grouped by the row-32 column k:
            #   ans_q = min_k ( zb_q[k] + min(zf_q[k], zf_q[k-1]) )
            # (down edge k->k, diag edge k-1->k; zf[-1] reads a guard, so the
            # nonexistent edge never wins). The cc ops read only chain A,
            # which the scheduler sequences first, so every seam dependency
            # is >=2 instructions back and the DVE runs the seam gap-free.
            # B chains stored col-flipped (sample q = slot 1-q, reversed).
            zfs = (za[:, 1:1 + W], za[:, 1 + SLOT:1 + SLOT + W])
            zfp = (za[:, 0:W], za[:, SLOT:SLOT + W])         # zf[k-1] views
            zbr = (zb[:, CW - 1:SLOT:-1], zb[:, SLOT - 1:0:-1])
            for q in range(Q):
                nc.vector.tensor_tensor(out=cc[:, q, :], in0=zfs[q],
                                        in1=zfp[q], op=MIN)
            for q in range(Q):
                nc.vector.tensor_tensor(out=t2[:, q, :], in0=cc[:, q, :],
                                        in1=zbr[q], op=ADD)
            for q in range(Q):
                nc.vector.tensor_reduce(out=red[:, q:q + 1], in_=t2[:, q, :],
                                        axis=mybir.AxisListType.X, op=MIN)
            nc.sync.dma_start(out=out_d, in_=red[:])
    nc.compile()
    return nc


def get_nc():
    if "nc" not in _CACHE:
        _CACHE["nc"] = _build()
    return _CACHE["nc"]


def _pack(images):
    """[2048,64,64] f32 -> per-core [8][128, 32, 260] f16 host-side."""
    img16 = np.ascontiguousarray(images, dtype=np.float16)
    blocks = img16.reshape(N_CORES, Q, P, H, W)
    out = np.full((N_CORES, P, HH, L), GUARD, dtype=np.float16)
    for c in range(N_CORES):
        s0 = blocks[c, 0]
        s1 = blocks[c, 1]
        out[c, :, :, 0 * SLOT:0 * SLOT + W] = s0[:, :HH, :]
        out[c, :, :, 1 * SLOT:1 * SLOT + W] = s1[:, :HH, :]
        out[c, :, :, 2 * SLOT:2 * SLOT + W] = np.flip(s1, axis=(1, 2))[:, :HH, :]
        out[c, :, :, 3 * SLOT:3 * SLOT + W] = np.flip(s0, axis=(1, 2))[:, :HH, :]
    return out


def kernel(images: np.ndarray, **run_kwargs) -> np.ndarray:
    B = images.shape[0]
    assert images.shape == (B, H, W) and B == N_CORES * NB_CORE
    packed = _pack(images)
    nc = get_nc()
    in_maps = [{"images": packed[c]} for c in range(N_CORES)]
    # the backend rarely hits a transient first-run execution fault that
    # succeeds on retry; one retry costs nothing when the run is healthy
    try:
        res = run_bass_kernel_spmd(nc, in_maps,
                                   core_ids=list(range(N_CORES)),
                                   **run_kwargs)
    except Exception:
        res = run_bass_kernel_spmd(nc, in_maps,
                                   core_ids=list(range(N_CORES)),
                                   **run_kwargs)
    out = np.empty((B,), dtype=np.float32)
    for c in range(N_CORES):
        out[c * NB_CORE:(c + 1) * NB_CORE] = res.results[c]["out"].T.reshape(-1)
    if run_kwargs:
        return out, res
    return out
